# revision 11
# baseline (speedup 1.0000x reference)
"""DecoderRNN (LSTM + vocab projection) Trainium2 kernel.

Strategy: data-parallel over batch B=64 across 8 NeuronCores (8 examples
per core). Per core:
  1. indirect-DMA gather of caption embeddings (bf16), PE-transpose -> X.T
     (feature rows first, then token rows t-major)
  2. X-projection GEMM split in three row passes: p0 (steps 0-1) ahead of
     the recurrence, p1 (steps 2-11) right after step 1, p2 (steps 12-32)
     interleaved into the eltwise-stall windows of steps 2..9
  3. 33 sequential LSTM cell steps in transposed layout:
       gates.T = W_hh.T.T @ h.T accumulated into PSUM tiles split by gate
       group (i+f / g / o), each preloaded with its X-projection term via an
       identity matmul; eltwise split across ACT/DVE/GPSIMD so the tensor
       engine restarts as early as possible. h.T is written directly into
       the FC stationary layout.
  4. batched FC GEMM [256, 512] @ [512, 10240] + bias. The first token
     half (m=0, steps 1..16) is emitted in 512-column units interleaved
     into the PE stall windows of steps 17..32; the second half runs after
     the last step. Output DMA rotates across queues.
All matmuls take bf16 inputs with fp32 PSUM accumulation.
"""

import numpy as np
import ml_dtypes

import concourse.bass as bass
import concourse.tile as tile
from concourse import bacc, mybir
from concourse import bass_utils
from concourse.masks import make_identity
from concourse.tile_rust import add_dep_helper

BF16 = ml_dtypes.bfloat16

# Problem shape (hardcoded per the task contract).
B, T, E, H, V = 64, 32, 512, 512, 10000
NCORES = 8
BL = B // NCORES            # 8 examples per core
STEPS = T + 1               # 33 cell steps (features + 32 caption tokens)
FOURH = 4 * H               # 2048
P = 128
NJ = FOURH // P             # 16 gate-unit chunks
NK = H // P                 # 4 contraction chunks
TOKR = T * BL               # 256 token rows (t-major)
ROWS = TOKR + BL            # 264 = feature rows + token rows
VP = 10240                  # padded vocab (20 * 512)
NU = VP // 512              # 20 vocab units of 512 columns

N_WARM = 55                 # PE warmup matmuls bridging to the gather

f32 = mybir.dt.float32
bf16 = mybir.dt.bfloat16
i32 = mybir.dt.int32

# Gate pack order along 4H is (i, f, g, o) — PyTorch's native order, so the
# chain-critical i/f sigmoids can start while the g/o matmuls still stream.
_PERM = np.arange(4 * H)

# xproj row passes over xT rows (features at rows 0..7, token t at 8+8t):
# (row0, row1, step0, step1)
_PASSES = ((0, 16, 0, 2), (16, 96, 2, 12), (96, 264, 12, 33))


def _build_program():
    nc = bacc.Bacc(
        "TRN2",
        target_bir_lowering=False,
        debug=False,
        num_devices=NCORES,
    )

    x_feat = nc.dram_tensor("x_feat", [BL, E], bf16, kind="ExternalInput").ap()
    tok_idx = nc.dram_tensor("tok_idx", [TOKR, 1], i32, kind="ExternalInput").ap()
    embed_w = nc.dram_tensor("embed_w", [V, E], bf16, kind="ExternalInput").ap()
    w_ihT = nc.dram_tensor("w_ihT", [E, FOURH], bf16, kind="ExternalInput").ap()
    w_hhT = nc.dram_tensor("w_hhT", [H, FOURH], bf16, kind="ExternalInput").ap()
    bias_t = nc.dram_tensor("bias_t", [P, NJ], f32, kind="ExternalInput").ap()
    fc_wT = nc.dram_tensor("fc_wT", [H, VP], bf16, kind="ExternalInput").ap()
    fc_b_bc = nc.dram_tensor("fc_b_bc", [P, VP], bf16, kind="ExternalInput").ap()
    out = nc.dram_tensor("out", [BL, T, V], f32, kind="ExternalOutput").ap()

    with tile.TileContext(nc) as tc:
        _kernel_body(tc, x_feat, tok_idx, embed_w, w_ihT, w_hhT, bias_t,
                     fc_wT, fc_b_bc, out)

    nc.compile()
    return nc


def _kernel_body(tc, x_feat, tok_idx, embed_w, w_ihT, w_hhT, bias_t,
                 fc_wT, fc_b_bc, out):
    from contextlib import ExitStack
    ctx = ExitStack()
    nc = tc.nc

    # ---- persistent tiles (one bufs=1 pool, distinct tags per name) ----
    cp = ctx.enter_context(tc.tile_pool(name="const", bufs=1))
    wih_sb = cp.tile([P, NK * FOURH], bf16, name="wih_sb", tag="wih_sb")
    whh_sb = cp.tile([P, NK * FOURH], bf16, name="whh_sb", tag="whh_sb")
    fcw_sb = cp.tile([P, NK * VP], bf16, name="fcw_sb", tag="fcw_sb")
    fcb_sb = cp.tile([P, VP], bf16, name="fcb_sb", tag="fcb_sb")
    biast_sb = cp.tile([P, NJ], f32, name="biast_sb", tag="biast_sb")
    ident = cp.tile([P, P], bf16, name="ident", tag="ident")
    idx_sb = cp.tile([P, 2], i32, name="idx_sb", tag="idx_sb")
    xn0 = cp.tile([P, E], bf16, name="xn0", tag="xn0")
    xn1 = cp.tile([P, E], bf16, name="xn1", tag="xn1")
    xf = cp.tile([P, E], bf16, name="xf", tag="xf")
    xT = cp.tile([P, NK * ROWS], bf16, name="xT", tag="xT")
    xpT = cp.tile([P, STEPS * P], bf16, name="xpT", tag="xpT")
    hT = cp.tile([P, NK * TOKR], bf16, name="hT", tag="hT")
    h0T = cp.tile([P, NK * BL], bf16, name="h0T", tag="h0T")
    cst = cp.tile([P, NK * BL], f32, name="cst", tag="cst")

    ps = ctx.enter_context(tc.tile_pool(name="ps", bufs=2, space="PSUM"))
    sb = ctx.enter_context(tc.tile_pool(name="sb", bufs=3))

    # ---- load constants ----
    # Sync queue: small critical loads + whh (needed by step 1 ~16us), then
    # the fc weight tiles (gated on the gather being consumed so their HBM
    # flood cannot starve the head critical path).
    make_identity(nc, ident[:])
    nc.sync.dma_start(idx_sb[:].rearrange("p (c o) -> p c o", o=1),
                      tok_idx.rearrange("(c p) o -> p c o", p=P))
    nc.sync.dma_start(xf[:BL, :], x_feat[:, :])
    nc.sync.dma_start(whh_sb[:].rearrange("p (k f) -> p k f", k=NK),
                      w_hhT.rearrange("(k p) f -> p k f", p=P))
    nc.sync.dma_start(biast_sb[:], bias_t[:])
    # Scalar queue: wih (needed ~13us for xproj p0), fc bias broadcast.
    nc.scalar.dma_start(wih_sb[:].rearrange("p (k f) -> p k f", k=NK),
                        w_ihT.rearrange("(k p) f -> p k f", p=P))
    nc.scalar.dma_start(fcb_sb[:], fc_b_bc[:])

    # ---- embedding gather ----
    nc.gpsimd.indirect_dma_start(
        out=xn0[:], out_offset=None, in_=embed_w[:],
        in_offset=bass.IndirectOffsetOnAxis(ap=idx_sb[:, 0:1], axis=0))
    nc.gpsimd.indirect_dma_start(
        out=xn1[:], out_offset=None, in_=embed_w[:],
        in_offset=bass.IndirectOffsetOnAxis(ap=idx_sb[:, 1:2], axis=0))

    # PSUM budget is 8 banks x 2KB: fc0/fc1 (2 bufs each, also reused by
    # warmup + transposes) = 4 banks, xp = 2, g3 = 2.

    # ---- PE warm-up: junk matmuls so the HAM clock-gate opens while the
    # gather is in flight; sized to end roughly when gather data lands ----
    for wi in range(N_WARM):
        wps = ps.tile([P, 512], f32, name="wps", tag=f"fc{wi % 2}")
        nc.tensor.matmul(wps[:, :P], lhsT=ident[:], rhs=ident[:],
                         start=True, stop=True)

    # ---- transpose X -> X.T ----
    # xT row layout per k chunk: [features 0..7 | xn0 tokens 8..135 |
    # xn1 tokens 136..263]. xf/xn0 transposes up front; xn1's are emitted
    # after xproj p0 (they gate only pass p2).
    cp_engines = (nc.vector, nc.gpsimd)
    tp_n = 0

    def _transpose(src, n_r, k, dst_off):
        nonlocal tp_n
        pt = ps.tile([P, 512], bf16, name="pst", tag=f"fc{tp_n % 2}")
        nc.tensor.transpose(pt[:, :n_r], src[:n_r, k * P:(k + 1) * P],
                            ident[:n_r, :n_r])
        dst = xT[:, k * ROWS + dst_off: k * ROWS + dst_off + n_r]
        if tp_n % 2 == 0:
            r = nc.vector.tensor_copy(out=dst, in_=pt[:, :n_r])
        else:
            r = nc.scalar.copy(out=dst, in_=pt[:, :n_r])
        tp_n += 1
        return r

    for k in range(NK):
        _transpose(xf, BL, k, 0)
    for k in range(NK):
        _gate_inst = _transpose(xn0, P, k, BL)

    # ---- X projection GEMM helper ----
    # xpT[:, s*128 + j*8 + b] = (X @ W_ihT)[row(s,b), j*128+p] + bias[j*128+p]
    xp_view = xpT[:].rearrange("p (s j b) -> p s j b", s=STEPS, j=NJ, b=BL)
    xp_add_n = 0

    def _xproj_j(j, pss):
        nonlocal xp_add_n
        r0, r1, s0, s1 = _PASSES[pss]
        w = r1 - r0
        pxp = ps.tile([P, 176], f32, name="pxp", tag="xp")
        for k in range(NK):
            nc.tensor.matmul(
                pxp[:, :w],
                lhsT=wih_sb[:, k * FOURH + j * P: k * FOURH + (j + 1) * P],
                rhs=xT[:, k * ROWS + r0:k * ROWS + r1],
                start=(k == 0), stop=(k == NK - 1))
        dst = xp_view[:, s0:s1, j, :]
        src = pxp[:, :w].rearrange("p (s b) -> p s b", b=BL)
        if xp_add_n % 2 == 0:
            nc.vector.tensor_scalar_add(dst, src, biast_sb[:, j:j + 1])
        else:
            # ACT identity-with-bias: PSUM-reading add off the DVE queue
            nc.scalar.activation(dst, src,
                                 mybir.ActivationFunctionType.Identity,
                                 bias=biast_sb[:, j:j + 1])
        xp_add_n += 1

    # pass p0 (steps 0-1) fully ahead of the recurrence
    for j in range(NJ):
        _xproj_j(j, 0)

    # ---- FC setup ----
    # fc weights stream as (vocab-unit, k) tiles on the sync queue, gated on
    # the last xn0 transpose copy so they start after the head's hot phase.
    for n in range(NU):
        for k in range(NK):
            fd = nc.sync.dma_start(
                fcw_sb[:, k * VP + n * 512: k * VP + (n + 1) * 512],
                fc_wT[k * P:(k + 1) * P, n * 512:(n + 1) * 512])
            if n == 0 and k == 0 and _gate_inst is not None:
                add_dep_helper(fd.ins, _gate_inst.ins, sync=True,
                               reason="delay fc weight stream past head")

    out_v = out[:, :, :]   # [BL, T, V]
    fc_stg = {}
    fc_dma_n = 0

    def _fc_unit(m, n, post):
        """One FC unit: identity matmul preloads the vocab bias into a
        [P,512] psum tile, 4 k-matmuls accumulate on top, then a plain
        PSUM->SBUF copy into a [P,1024] staging pair; DMA out when the
        pair completes."""
        nonlocal fc_dma_n
        pfc = ps.tile([P, 512], f32, name="pfc", tag=f"fc{n % 2}")
        nc.tensor.matmul(pfc[:], lhsT=ident[:],
                         rhs=fcb_sb[:, n * 512:(n + 1) * 512],
                         start=True, stop=False, skip_group_check=True)
        for k in range(NK):
            nc.tensor.matmul(
                pfc[:],
                lhsT=hT[:, k * TOKR + m * P: k * TOKR + (m + 1) * P],
                rhs=fcw_sb[:, k * VP + n * 512: k * VP + (n + 1) * 512],
                start=False, stop=(k == NK - 1), skip_group_check=True)
        pair = n // 2
        if n % 2 == 0:
            fc_stg[(m, pair)] = sb.tile([P, 1024], f32, name="stg", tag="stg")
        stg = fc_stg[(m, pair)]
        dst = stg[:, (n % 2) * 512:(n % 2) * 512 + 512]
        if (m * NU + n) % 2 == 0:
            nc.vector.tensor_copy(out=dst, in_=pfc[:])
        else:
            nc.scalar.copy(out=dst, in_=pfc[:])
        if n % 2 == 1:
            vlo = (pair * 1024)
            gw = min(V, vlo + 1024) - vlo
            qeng = (nc.sync, nc.scalar)[fc_dma_n % 2]
            fc_dma_n += 1
            if gw > 0:
                qeng.dma_start(
                    out=out_v[:, m * 16:(m + 1) * 16, vlo:vlo + gw]
                    .rearrange("b t v -> t b v"),
                    in_=stg[:, :gw])

    # ---- recurrence ----
    hT_view = hT[:].rearrange("p (k s b) -> p k s b", k=NK, s=T, b=BL)
    h0_view = h0T[:].rearrange("p (k b) -> p k b", k=NK)

    def _hprev(c, k):
        if c == 1:
            return h0T[:, k * BL:(k + 1) * BL]
        off = k * TOKR + (c - 2) * BL
        return hT[:, off: off + BL]

    # Gate groups: (name, j-range, xp column offset, width) in (if, g, o) order
    GRP = (("gif", 0, 8, 0, 64), ("gg", 8, 12, 64, 32), ("go", 12, 16, 96, 32))

    for c in range(STEPS):
        if c == 0:
            g_if, g_g, g_o = (xpT[:, 0:64], xpT[:, 64:96], xpT[:, 96:128])
        else:
            pg3 = ps.tile([P, P], f32, name="pg3", tag="g3")
            # single identity matmul preloads the whole step's PSUM with the
            # X-projection terms (start=True sets has_written so the W
            # matmuls accumulate on top)
            nc.tensor.matmul(
                pg3[:], lhsT=ident[:], rhs=xpT[:, c * P:(c + 1) * P],
                start=True, stop=False, skip_group_check=True)
            for (tag, j0, j1, xoff, wdt) in GRP:
                for k in range(NK):
                    for j in range(j0, j1):
                        col = xoff + (j - j0) * BL
                        nc.tensor.matmul(
                            pg3[:, col:col + BL],
                            lhsT=whh_sb[:, k * FOURH + j * P: k * FOURH + (j + 1) * P],
                            rhs=_hprev(c, k),
                            start=False,
                            stop=(tag == "go" and j == j1 - 1 and k == NK - 1),
                            skip_group_check=True)
            g_if, g_g, g_o = pg3[:, 0:64], pg3[:, 64:96], pg3[:, 96:128]

        act_g = sb.tile([P, 32], f32, name="act_g")
        act_if = sb.tile([P, 64], f32, name="act_if")
        act_o = sb.tile([P, 32], f32, name="act_o")
        nc.scalar.activation(act_if[:], g_if,
                             mybir.ActivationFunctionType.Sigmoid)
        nc.scalar.activation(act_g[:], g_g,
                             mybir.ActivationFunctionType.Tanh)
        nc.scalar.activation(act_o[:], g_o,
                             mybir.ActivationFunctionType.Sigmoid)

        if c == 0:
            # c_new = i * g  (previous c is zero)
            nc.vector.tensor_mul(out=cst[:], in0=act_if[:, 0:32], in1=act_g[:])
        else:
            ig = sb.tile([P, 32], f32, name="ig")
            fc2 = sb.tile([P, 32], f32, name="fc2")
            nc.gpsimd.tensor_mul(out=fc2[:], in0=act_if[:, 32:64], in1=cst[:])
            nc.vector.tensor_mul(out=ig[:], in0=act_if[:, 0:32], in1=act_g[:])
            nc.vector.tensor_add(out=cst[:], in0=ig[:], in1=fc2[:])

        tch = sb.tile([P, 32], f32, name="tch")
        nc.scalar.activation(tch[:], cst[:], mybir.ActivationFunctionType.Tanh)

        if c == 0:
            hdst = h0_view
        else:
            hdst = hT_view[:, :, c - 1, :]
        o_v = act_o[:].rearrange("p (k b) -> p k b", k=NK)
        t_v = tch[:].rearrange("p (k b) -> p k b", k=NK)
        nc.vector.tensor_mul(out=hdst[:, 0:2, :], in0=o_v[:, 0:2, :],
                             in1=t_v[:, 0:2, :])
        nc.gpsimd.tensor_mul(out=hdst[:, 2:4, :], in0=o_v[:, 2:4, :],
                             in1=t_v[:, 2:4, :])

        # ---- stall-window fillers (run on the PE while the eltwise chain
        # of step c completes) ----
        if c == 0:
            for k in range(NK):
                _transpose(xn1, P, k, BL + P)
        elif c == 1:
            for j in range(NJ):
                _xproj_j(j, 1)          # xproj p1: steps 2..11
        elif 2 <= c <= 9:
            for j in (2 * (c - 2), 2 * (c - 2) + 1):
                _xproj_j(j, 2)          # xproj p2: steps 12..32
        elif 17 <= c <= 28:
            _fc_unit(0, c - 17, post=False)
        elif 29 <= c <= 32:
            _fc_unit(0, 12 + 2 * (c - 29), post=False)
            _fc_unit(0, 13 + 2 * (c - 29), post=False)

    # ---- FC second token half (m=1) + remaining output DMA ----
    for n in range(NU):
        _fc_unit(1, n, post=True)
    ctx.close()


_NC_CACHE = {}


def _get_program():
    if "nc" not in _NC_CACHE:
        _NC_CACHE["nc"] = _build_program()
    return _NC_CACHE["nc"]


def make_in_maps(features, captions, embed_W, W_ih, W_hh, b_ih, b_hh, fc_W, fc_b):
    """Host-side sharding + layout prep. Pure layout/dtype work, no math
    beyond summing the two bias vectors."""
    embed_bf = embed_W.astype(BF16)
    w_ihT = np.ascontiguousarray(W_ih.T[:, _PERM]).astype(BF16)
    w_hhT = np.ascontiguousarray(W_hh.T[:, _PERM]).astype(BF16)
    bias = (b_ih + b_hh).astype(np.float32)[_PERM]
    bias_t = np.ascontiguousarray(bias.reshape(NJ, P).T)
    fc_wT = np.zeros((H, VP), dtype=BF16)
    fc_wT[:, :V] = fc_W.T.astype(BF16)
    fcb = np.zeros((VP,), dtype=BF16)
    fcb[:V] = fc_b.astype(BF16)
    fc_b_bc = np.ascontiguousarray(np.broadcast_to(fcb, (P, VP)))

    in_maps = []
    for core in range(NCORES):
        sl = slice(core * BL, (core + 1) * BL)
        cap = captions[sl].astype(np.int32)          # [BL, T]
        tok = np.ascontiguousarray(cap.T).reshape(TOKR, 1)  # t-major
        in_maps.append({
            "x_feat": np.ascontiguousarray(features[sl]).astype(BF16),
            "tok_idx": tok,
            "embed_w": embed_bf,
            "w_ihT": w_ihT,
            "w_hhT": w_hhT,
            "bias_t": bias_t,
            "fc_wT": fc_wT,
            "fc_b_bc": fc_b_bc,
        })
    return in_maps


def _ensure_ntff_hook():
    """The agent image's antenv package lacks axon_hooks; synthesize it so
    run_bass_kernel_spmd(trace=True) can capture NTFF profiles."""
    import sys
    import types
    try:
        from antenv.axon_hooks import get_axon_ntff_profile_hook  # noqa: F401
        return
    except ImportError:
        pass
    import antenv
    mod = types.ModuleType("antenv.axon_hooks")
    state = {}
    mod.set_axon_ntff_profile_hook = lambda h: state.__setitem__("h", h)
    mod.get_axon_ntff_profile_hook = lambda: state.get("h")
    sys.modules["antenv.axon_hooks"] = mod
    antenv.axon_hooks = mod
    try:
        from trn_agent_boot.trn_boot import _ntff_profile_via_ctypes
        hook = _ntff_profile_via_ctypes("/opt/axon/libaxon_pjrt.so")
        if hook is not None:
            mod.set_axon_ntff_profile_hook(hook)
    except Exception as e:  # degrade: tracing skipped, run still works
        print(f"ntff hook setup failed: {e}")


def kernel(features, captions, embed_W, W_ih, W_hh, b_ih, b_hh, fc_W, fc_b,
           _trace=False):
    nc = _get_program()
    in_maps = make_in_maps(features, captions, embed_W, W_ih, W_hh,
                           b_ih, b_hh, fc_W, fc_b)
    if _trace:
        _ensure_ntff_hook()
    res = bass_utils.run_bass_kernel_spmd(
        nc, in_maps, core_ids=list(range(NCORES)), trace=_trace)
    out = np.concatenate([res.results[c]["out"] for c in range(NCORES)], axis=0)
    if _trace:
        kernel.last_result = res
    return out


# revision 17
# speedup vs baseline: 1.0522x; 1.0522x over previous
"""DecoderRNN (LSTM + vocab projection) Trainium2 kernel.

Strategy: data-parallel over batch B=64 across 8 NeuronCores (8 examples
per core). Per core:
  1. indirect-DMA gather of caption embeddings (bf16), PE-transpose -> X.T
     (feature rows first, then token rows t-major)
  2. X-projection GEMM split in three row passes: p0 (steps 0-1) ahead of
     the recurrence, p1 (steps 2-11) right after step 1, p2 (steps 12-32)
     interleaved into the eltwise-stall windows of steps 2..9
  3. 33 sequential LSTM cell steps in transposed layout:
       gates.T = W_hh.T.T @ h.T accumulated into PSUM tiles split by gate
       group (i+f / g / o), each preloaded with its X-projection term via an
       identity matmul; eltwise split across ACT/DVE/GPSIMD so the tensor
       engine restarts as early as possible. h.T is written directly into
       the FC stationary layout.
  4. batched FC GEMM [256, 512] @ [512, 10240] + bias. The first token
     half (m=0, steps 1..16) is emitted in 512-column units interleaved
     into the PE stall windows of steps 17..32; the second half runs after
     the last step. Output DMA rotates across queues.
All matmuls take bf16 inputs with fp32 PSUM accumulation.
"""

import numpy as np
import ml_dtypes

import concourse.bass as bass
import concourse.tile as tile
from concourse import bacc, mybir
from concourse import bass_utils
from concourse.masks import make_identity
from concourse.tile_rust import add_dep_helper

BF16 = ml_dtypes.bfloat16

# Problem shape (hardcoded per the task contract).
B, T, E, H, V = 64, 32, 512, 512, 10000
NCORES = 8
BL = B // NCORES            # 8 examples per core
STEPS = T + 1               # 33 cell steps (features + 32 caption tokens)
FOURH = 4 * H               # 2048
P = 128
NJ = FOURH // P             # 16 gate-unit chunks
NK = H // P                 # 4 contraction chunks
TOKR = T * BL               # 256 token rows (t-major)
ROWS = TOKR + BL            # 264 = feature rows + token rows
VP = 10240                  # padded vocab (20 * 512)
NU = VP // 512              # 20 vocab units of 512 columns

N_WARM = 68                 # PE warmup matmuls bridging to the gather

f32 = mybir.dt.float32
bf16 = mybir.dt.bfloat16
i32 = mybir.dt.int32

# Gate pack order along 4H is (i, f, g, o) — PyTorch's native order, so the
# chain-critical i/f sigmoids can start while the g/o matmuls still stream.
_PERM = np.arange(4 * H)

# xproj row passes over xT rows (features at rows 0..7, token t at 8+8t):
# (row0, row1, step0, step1)
_PASSES = ((0, 16, 0, 2), (16, 96, 2, 12), (96, 264, 12, 33))


def _build_program():
    nc = bacc.Bacc(
        "TRN2",
        target_bir_lowering=False,
        debug=False,
        num_devices=NCORES,
    )

    x_feat = nc.dram_tensor("x_feat", [BL, E], bf16, kind="ExternalInput").ap()
    tok_idx = nc.dram_tensor("tok_idx", [TOKR, 1], i32, kind="ExternalInput").ap()
    embed_w = nc.dram_tensor("embed_w", [V, E], bf16, kind="ExternalInput").ap()
    w_ihT = nc.dram_tensor("w_ihT", [E, FOURH], bf16, kind="ExternalInput").ap()
    w_hhT = nc.dram_tensor("w_hhT", [H, FOURH], bf16, kind="ExternalInput").ap()
    bias_t = nc.dram_tensor("bias_t", [P, NJ], f32, kind="ExternalInput").ap()
    fc_wT = nc.dram_tensor("fc_wT", [H, VP], bf16, kind="ExternalInput").ap()
    fc_b_bc = nc.dram_tensor("fc_b_bc", [P, VP], bf16, kind="ExternalInput").ap()
    out = nc.dram_tensor("out", [BL, T, V], f32, kind="ExternalOutput").ap()

    with tile.TileContext(nc) as tc:
        _kernel_body(tc, x_feat, tok_idx, embed_w, w_ihT, w_hhT, bias_t,
                     fc_wT, fc_b_bc, out)

    nc.compile()
    return nc


def _kernel_body(tc, x_feat, tok_idx, embed_w, w_ihT, w_hhT, bias_t,
                 fc_wT, fc_b_bc, out):
    from contextlib import ExitStack
    ctx = ExitStack()
    nc = tc.nc

    # ---- persistent tiles (one bufs=1 pool, distinct tags per name) ----
    cp = ctx.enter_context(tc.tile_pool(name="const", bufs=1))
    wih_sb = cp.tile([P, NK * FOURH], bf16, name="wih_sb", tag="wih_sb")
    whh_sb = cp.tile([P, NK * FOURH], bf16, name="whh_sb", tag="whh_sb")
    fcw_sb = cp.tile([P, NK * VP], bf16, name="fcw_sb", tag="fcw_sb")
    fcb_sb = cp.tile([P, VP], bf16, name="fcb_sb", tag="fcb_sb")
    biast_sb = cp.tile([P, NJ], f32, name="biast_sb", tag="biast_sb")
    ident = cp.tile([P, P], bf16, name="ident", tag="ident")
    idx_sb = cp.tile([P, 2], i32, name="idx_sb", tag="idx_sb")
    xn0 = cp.tile([P, E], bf16, name="xn0", tag="xn0")
    xn1 = cp.tile([P, E], bf16, name="xn1", tag="xn1")
    xf = cp.tile([P, E], bf16, name="xf", tag="xf")
    xT = cp.tile([P, NK * ROWS], bf16, name="xT", tag="xT")
    xpT = cp.tile([P, STEPS * P], bf16, name="xpT", tag="xpT")
    hT = cp.tile([P, NK * TOKR], bf16, name="hT", tag="hT")
    h0T = cp.tile([P, NK * BL], bf16, name="h0T", tag="h0T")
    cst = cp.tile([P, NK * BL], f32, name="cst", tag="cst")

    ps = ctx.enter_context(tc.tile_pool(name="ps", bufs=2, space="PSUM"))
    sb = ctx.enter_context(tc.tile_pool(name="sb", bufs=3))

    # ---- load constants ----
    # Sync queue: small critical loads + whh (needed by step 1 ~16us), then
    # the fc weight tiles (gated on the gather being consumed so their HBM
    # flood cannot starve the head critical path).
    make_identity(nc, ident[:])
    nc.sync.dma_start(idx_sb[:].rearrange("p (c o) -> p c o", o=1),
                      tok_idx.rearrange("(c p) o -> p c o", p=P))
    nc.sync.dma_start(xf[:BL, :], x_feat[:, :])
    nc.sync.dma_start(whh_sb[:].rearrange("p (k f) -> p k f", k=NK),
                      w_hhT.rearrange("(k p) f -> p k f", p=P))
    nc.sync.dma_start(biast_sb[:], bias_t[:])
    # Scalar queue: wih (needed ~13us for xproj p0), fc bias broadcast.
    nc.scalar.dma_start(wih_sb[:].rearrange("p (k f) -> p k f", k=NK),
                        w_ihT.rearrange("(k p) f -> p k f", p=P))
    nc.scalar.dma_start(fcb_sb[:], fc_b_bc[:])

    # ---- embedding gather ----
    nc.gpsimd.indirect_dma_start(
        out=xn0[:], out_offset=None, in_=embed_w[:],
        in_offset=bass.IndirectOffsetOnAxis(ap=idx_sb[:, 0:1], axis=0))
    nc.gpsimd.indirect_dma_start(
        out=xn1[:], out_offset=None, in_=embed_w[:],
        in_offset=bass.IndirectOffsetOnAxis(ap=idx_sb[:, 1:2], axis=0))

    # ---- prime the ACT function tables while the runtime preamble is
    # still settling, so no table reload lands mid-kernel ----
    prime = cp.tile([P, 4], f32, name="prime", tag="prime")
    nc.scalar.activation(prime[:, 0:1], ident[:, 0:1],
                         mybir.ActivationFunctionType.Sigmoid)
    nc.scalar.activation(prime[:, 1:2], ident[:, 0:1],
                         mybir.ActivationFunctionType.Tanh)
    nc.scalar.activation(prime[:, 2:3], ident[:, 0:1],
                         mybir.ActivationFunctionType.Identity,
                         bias=biast_sb[:, 0:1])

    # ---- PE warm-up: junk matmuls so the HAM clock-gate opens while the
    # gather is in flight; sized to end roughly when gather data lands ----
    for wi in range(N_WARM):
        wps = ps.tile([P, 512], f32, name="wps", tag="fc0")
        nc.tensor.matmul(wps[:, :P], lhsT=ident[:], rhs=ident[:],
                         start=True, stop=True)

    # ---- transpose X -> X.T ----
    # xT row layout per k chunk: [features 0..7 | xn0 tokens 8..135 |
    # xn1 tokens 136..263]. All 4 k-chunks of one source transpose into one
    # PSUM tile, then a single strided copy writes them into xT.
    xT_k = xT[:].rearrange("p (k r) -> p k r", k=NK)
    tp_n = 0

    def _transpose_src(src, n_r, dst_off):
        nonlocal tp_n
        pt = ps.tile([P, 512], bf16, name="pst", tag="fc0")
        for k in range(NK):
            nc.tensor.transpose(pt[:, k * P:k * P + n_r],
                                src[:n_r, k * P:(k + 1) * P],
                                ident[:n_r, :n_r])
        dst = xT_k[:, :, dst_off:dst_off + n_r]
        pt_v = pt[:].rearrange("p (k r) -> p k r", k=NK)[:, :, :n_r]
        if tp_n % 2 == 0:
            r = nc.vector.tensor_copy(out=dst, in_=pt_v)
        else:
            r = nc.scalar.copy(out=dst, in_=pt_v)
        tp_n += 1
        return r

    _transpose_src(xf, BL, 0)
    _gate_inst = _transpose_src(xn0, P, BL)

    # ---- X projection GEMM helper ----
    # xpT[:, s*128 + j*8 + b] = (X @ W_ihT)[row(s,b), j*128+p] + bias[j*128+p]
    xp_view = xpT[:].rearrange("p (s j b) -> p s j b", s=STEPS, j=NJ, b=BL)
    xp_add_n = 0

    def _xproj_j(j, pss):
        nonlocal xp_add_n
        r0, r1, s0, s1 = _PASSES[pss]
        w = r1 - r0
        pxp = ps.tile([P, 176], f32, name="pxp", tag="fc0")
        for k in range(NK):
            nc.tensor.matmul(
                pxp[:, :w],
                lhsT=wih_sb[:, k * FOURH + j * P: k * FOURH + (j + 1) * P],
                rhs=xT[:, k * ROWS + r0:k * ROWS + r1],
                start=(k == 0), stop=(k == NK - 1))
        dst = xp_view[:, s0:s1, j, :]
        src = pxp[:, :w].rearrange("p (s b) -> p s b", b=BL)
        if xp_add_n % 2 == 0:
            nc.vector.tensor_scalar_add(dst, src, biast_sb[:, j:j + 1])
        else:
            # ACT identity-with-bias: PSUM-reading add off the DVE queue
            nc.scalar.activation(dst, src,
                                 mybir.ActivationFunctionType.Identity,
                                 bias=biast_sb[:, j:j + 1])
        xp_add_n += 1

    # pass p0 (steps 0-1) fully ahead of the recurrence
    for j in range(NJ):
        _xproj_j(j, 0)

    # ---- FC setup ----
    # fc weights stream as (vocab-unit, k) tiles on the sync queue, gated on
    # the last xn0 transpose copy so they start after the head's hot phase.
    for n in range(NU):
        for k in range(NK):
            fd = nc.sync.dma_start(
                fcw_sb[:, k * VP + n * 512: k * VP + (n + 1) * 512],
                fc_wT[k * P:(k + 1) * P, n * 512:(n + 1) * 512])
            if n == 0 and k == 0 and _gate_inst is not None:
                add_dep_helper(fd.ins, _gate_inst.ins, sync=True,
                               reason="delay fc weight stream past head")

    out_v = out[:, :, :]   # [BL, T, V]
    fc_stg = {}
    fc_dma_n = 0

    def _fc_unit(m, n, post):
        """One FC unit: identity matmul preloads the vocab bias into a
        [P,512] psum tile, 4 k-matmuls accumulate on top, then a plain
        PSUM->SBUF copy into a [P,1024] staging pair; DMA out when the
        pair completes."""
        nonlocal fc_dma_n
        pfc = ps.tile([P, 512], f32, name="pfc", tag="fc0")
        nc.tensor.matmul(pfc[:], lhsT=ident[:],
                         rhs=fcb_sb[:, n * 512:(n + 1) * 512],
                         start=True, stop=False, skip_group_check=True)
        for k in range(NK):
            nc.tensor.matmul(
                pfc[:],
                lhsT=hT[:, k * TOKR + m * P: k * TOKR + (m + 1) * P],
                rhs=fcw_sb[:, k * VP + n * 512: k * VP + (n + 1) * 512],
                start=False, stop=(k == NK - 1), skip_group_check=True)
        pair = n // 2
        if n % 2 == 0:
            fc_stg[(m, pair)] = sb.tile([P, 1024], f32, name="stg", tag="stg")
        stg = fc_stg[(m, pair)]
        dst = stg[:, (n % 2) * 512:(n % 2) * 512 + 512]
        if (m * NU + n) % 2 == 0:
            nc.vector.tensor_copy(out=dst, in_=pfc[:])
        else:
            nc.scalar.copy(out=dst, in_=pfc[:])
        if n % 2 == 1:
            vlo = (pair * 1024)
            gw = min(V, vlo + 1024) - vlo
            qeng = (nc.sync, nc.scalar)[fc_dma_n % 2]
            fc_dma_n += 1
            if gw > 0:
                qeng.dma_start(
                    out=out_v[:, m * 16:(m + 1) * 16, vlo:vlo + gw]
                    .rearrange("b t v -> t b v"),
                    in_=stg[:, :gw])

    # ---- recurrence ----
    hT_view = hT[:].rearrange("p (k s b) -> p k s b", k=NK, s=T, b=BL)
    h0_view = h0T[:].rearrange("p (k b) -> p k b", k=NK)

    def _hprev(c, k):
        if c == 1:
            return h0T[:, k * BL:(k + 1) * BL]
        off = k * TOKR + (c - 2) * BL
        return hT[:, off: off + BL]

    # Gate groups: (name, j-range, xp column offset, width) in (if, g, o) order
    GRP = (("gif", 0, 8, 0, 64), ("gg", 8, 12, 64, 32), ("go", 12, 16, 96, 32))

    for c in range(STEPS):
        if c == 0:
            g_if, g_g, g_o = (xpT[:, 0:64], xpT[:, 64:96], xpT[:, 96:128])
        else:
            # one PSUM tile per gate group so each group's ACT can start as
            # soon as its own matmuls stop (a shared tile serializes reads
            # behind the whole step's accumulation)
            tiles = {}
            for (tag, j0, j1, xoff, wdt) in GRP:
                pg = ps.tile([P, 64], f32, name=tag, tag=tag)[:, :wdt]
                tiles[tag] = pg
                # identity matmul preloads PSUM with the X-projection term
                # (start=True sets has_written so W matmuls accumulate)
                nc.tensor.matmul(
                    pg, lhsT=ident[:], rhs=xpT[:, c * P + xoff: c * P + xoff + wdt],
                    start=True, stop=False, skip_group_check=True)
                for k in range(NK):
                    for j in range(j0, j1):
                        nc.tensor.matmul(
                            pg[:, (j - j0) * BL:(j - j0 + 1) * BL],
                            lhsT=whh_sb[:, k * FOURH + j * P: k * FOURH + (j + 1) * P],
                            rhs=_hprev(c, k),
                            start=False, stop=(j == j1 - 1 and k == NK - 1),
                            skip_group_check=True)
            g_g, g_if, g_o = tiles["gg"], tiles["gif"], tiles["go"]

        act_g = sb.tile([P, 32], f32, name="act_g")
        act_if = sb.tile([P, 64], f32, name="act_if")
        act_o = sb.tile([P, 32], f32, name="act_o")
        nc.scalar.activation(act_if[:], g_if,
                             mybir.ActivationFunctionType.Sigmoid)
        nc.scalar.activation(act_g[:], g_g,
                             mybir.ActivationFunctionType.Tanh)
        nc.scalar.activation(act_o[:], g_o,
                             mybir.ActivationFunctionType.Sigmoid)

        if c == 0:
            # c_new = i * g  (previous c is zero)
            nc.vector.tensor_mul(out=cst[:], in0=act_if[:, 0:32], in1=act_g[:])
        else:
            ig = sb.tile([P, 32], f32, name="ig")
            fc2 = sb.tile([P, 32], f32, name="fc2")
            nc.gpsimd.tensor_mul(out=fc2[:], in0=act_if[:, 32:64], in1=cst[:])
            nc.vector.tensor_mul(out=ig[:], in0=act_if[:, 0:32], in1=act_g[:])
            nc.vector.tensor_add(out=cst[:], in0=ig[:], in1=fc2[:])

        tch = sb.tile([P, 32], f32, name="tch")
        nc.scalar.activation(tch[:], cst[:], mybir.ActivationFunctionType.Tanh)

        if c == 0:
            hdst = h0_view
        else:
            hdst = hT_view[:, :, c - 1, :]
        o_v = act_o[:].rearrange("p (k b) -> p k b", k=NK)
        t_v = tch[:].rearrange("p (k b) -> p k b", k=NK)
        nc.vector.tensor_mul(out=hdst[:, 0:2, :], in0=o_v[:, 0:2, :],
                             in1=t_v[:, 0:2, :])
        nc.gpsimd.tensor_mul(out=hdst[:, 2:4, :], in0=o_v[:, 2:4, :],
                             in1=t_v[:, 2:4, :])

        # ---- stall-window fillers (run on the PE while the eltwise chain
        # of step c completes) ----
        if c == 0:
            _transpose_src(xn1, P, BL + P)
        elif c == 1:
            for j in range(NJ):
                _xproj_j(j, 1)          # xproj p1: steps 2..11
        elif 2 <= c <= 9:
            for j in (2 * (c - 2), 2 * (c - 2) + 1):
                _xproj_j(j, 2)          # xproj p2: steps 12..32
        elif 17 <= c <= 28:
            _fc_unit(0, c - 17, post=False)
        elif 29 <= c <= 32:
            _fc_unit(0, 12 + 2 * (c - 29), post=False)
            _fc_unit(0, 13 + 2 * (c - 29), post=False)

    # ---- FC second token half (m=1) + remaining output DMA ----
    for n in range(NU):
        _fc_unit(1, n, post=True)
    ctx.close()


_NC_CACHE = {}


def _get_program():
    if "nc" not in _NC_CACHE:
        _NC_CACHE["nc"] = _build_program()
    return _NC_CACHE["nc"]


def make_in_maps(features, captions, embed_W, W_ih, W_hh, b_ih, b_hh, fc_W, fc_b):
    """Host-side sharding + layout prep. Pure layout/dtype work, no math
    beyond summing the two bias vectors."""
    embed_bf = embed_W.astype(BF16)
    w_ihT = np.ascontiguousarray(W_ih.T[:, _PERM]).astype(BF16)
    w_hhT = np.ascontiguousarray(W_hh.T[:, _PERM]).astype(BF16)
    bias = (b_ih + b_hh).astype(np.float32)[_PERM]
    bias_t = np.ascontiguousarray(bias.reshape(NJ, P).T)
    fc_wT = np.zeros((H, VP), dtype=BF16)
    fc_wT[:, :V] = fc_W.T.astype(BF16)
    fcb = np.zeros((VP,), dtype=BF16)
    fcb[:V] = fc_b.astype(BF16)
    fc_b_bc = np.ascontiguousarray(np.broadcast_to(fcb, (P, VP)))

    in_maps = []
    for core in range(NCORES):
        sl = slice(core * BL, (core + 1) * BL)
        cap = captions[sl].astype(np.int32)          # [BL, T]
        tok = np.ascontiguousarray(cap.T).reshape(TOKR, 1)  # t-major
        in_maps.append({
            "x_feat": np.ascontiguousarray(features[sl]).astype(BF16),
            "tok_idx": tok,
            "embed_w": embed_bf,
            "w_ihT": w_ihT,
            "w_hhT": w_hhT,
            "bias_t": bias_t,
            "fc_wT": fc_wT,
            "fc_b_bc": fc_b_bc,
        })
    return in_maps


def _ensure_ntff_hook():
    """The agent image's antenv package lacks axon_hooks; synthesize it so
    run_bass_kernel_spmd(trace=True) can capture NTFF profiles."""
    import sys
    import types
    try:
        from antenv.axon_hooks import get_axon_ntff_profile_hook  # noqa: F401
        return
    except ImportError:
        pass
    import antenv
    mod = types.ModuleType("antenv.axon_hooks")
    state = {}
    mod.set_axon_ntff_profile_hook = lambda h: state.__setitem__("h", h)
    mod.get_axon_ntff_profile_hook = lambda: state.get("h")
    sys.modules["antenv.axon_hooks"] = mod
    antenv.axon_hooks = mod
    try:
        from trn_agent_boot.trn_boot import _ntff_profile_via_ctypes
        hook = _ntff_profile_via_ctypes("/opt/axon/libaxon_pjrt.so")
        if hook is not None:
            mod.set_axon_ntff_profile_hook(hook)
    except Exception as e:  # degrade: tracing skipped, run still works
        print(f"ntff hook setup failed: {e}")


def kernel(features, captions, embed_W, W_ih, W_hh, b_ih, b_hh, fc_W, fc_b,
           _trace=False):
    nc = _get_program()
    in_maps = make_in_maps(features, captions, embed_W, W_ih, W_hh,
                           b_ih, b_hh, fc_W, fc_b)
    if _trace:
        _ensure_ntff_hook()
    res = bass_utils.run_bass_kernel_spmd(
        nc, in_maps, core_ids=list(range(NCORES)), trace=_trace)
    out = np.concatenate([res.results[c]["out"] for c in range(NCORES)], axis=0)
    if _trace:
        kernel.last_result = res
    return out
